# revision 1
# baseline (speedup 1.0000x reference)
"""Trainium2 Bass kernel for nn_CalibrationNetwork (MoE-routed 3-layer MLP + softmax).

Strategy (v2)
-------------
Host: sort samples by judge id, pad each judge group to a multiple of 256
("supertile"), distribute supertiles round-robin over 8 cores. Judge weights
are combined (W1+W1_a[j] etc.) on the host so the device never gathers.

Within a supertile the 256 samples form 128 even/odd PAIRS. All matmuls use
a block-diagonal parity layout: partition dim = (parity, feature/hidden),
free dim = (question, pair). All 7 questions are uniform (no pairing of
questions, unlike v1).

  L1 is an fp8 DoubleRow matmul (0.5 cycles/col): the two k-tiles carry a
  hi/lo residual split of W1 (W1 = fp8(W1) + fp8(W1 - fp8(W1))), so W1
  quantization error cancels; x is shifted by -0.5 before fp8 quantization
  (bias row absorbs the shift) which quarters the x quantization error.
  The moving x operand is reused across both k-tiles via a stride-0 AP dim.

  per supertile (layouts: p=(par,h) 128 partitions, cols=(q,pair)):
    L1: 7 DR matmuls  p1[:, q*128:+128] = W1blk_q^T(hi|lo) @ x_q   (64 cyc each)
    relu1 (DVE):  z1 = max(p1, 0) -> bf16 [128, 896]
    L2: 2 matmuls p2 = [[W2,0],[0,W2]]^T @ z1      (N=512+384, shared stationary)
    relu2 (ACT):  z2 = relu(p2 + b2) -> bf16
    L3: 1 bias matmul (ones^T @ b3row, K=1 N=70) + 7 matmuls
        p3[:, q*10:+10] += z2_q^T @ Vblk_q         (z2 stationary, N=10)
    out: copy p3 -> bf16 logits [128, 70] (DVE/ACT alternating), cols
         q*10 + par*5 + o; softmax runs on the HOST.

  DMA: per macro (2 supertiles): xw fp8 load + wb bf16 load on the SP HWDGE
  ring, one [128, 140] bf16 logits store on the GpSimd SWDGE queue (last
  store on SP to spare the exit drain).

The program is compiled per call for the actual supertile count (rounded to
whole macros); with the reference inputs that is 18 slots/core (9 macros).
"""

import numpy as np
import ml_dtypes

B, J, Q, O = 32768, 32, 7, 5
H = 64            # H1 == H2
ST = 256          # samples per supertile
NP = 128          # pairs per supertile
T = 20            # supertiles per core (worst case 32768/256 + 32 = 160 = 8*20)
N_CORES = 8
XWU = 896 + 7 * 256   # per-supertile xw cols: x [12,896] + W1 blocks [12,1792]
WBU = 272             # per-supertile wb cols (128 W2 + 70 V + 2 b2 + 70 b3 + 2 pad)

_bf16 = ml_dtypes.bfloat16
_f8 = ml_dtypes.float8_e4m3
_cache = {}


# ----------------------------------------------------------------------------
# device program
# ----------------------------------------------------------------------------

def _build_program(teff):
    import concourse.bacc as bacc
    import concourse.tile as tile
    import concourse.mybir as mybir
    import concourse.bass as bass
    from contextlib import ExitStack

    bf = mybir.dt.bfloat16
    f8 = mybir.dt.float8e4
    f32 = mybir.dt.float32
    AF = mybir.ActivationFunctionType
    DR = mybir.MatmulPerfMode.DoubleRow

    nc = bacc.Bacc("TRN2", target_bir_lowering=False, debug=False)
    nm = teff // 2
    xw_d = nc.dram_tensor("xw", (nm, 12, 2, XWU), f8, kind="ExternalInput")
    wb_d = nc.dram_tensor("wb", (nm, 128, 2 * WBU), bf, kind="ExternalInput")
    out_d = nc.dram_tensor("out", (nm, 128, 140), bf, kind="ExternalOutput")

    with ExitStack() as ctx:
        tc = ctx.enter_context(tile.TileContext(nc))
        cpool = ctx.enter_context(tc.tile_pool(name="const", bufs=1))
        inp = ctx.enter_context(tc.tile_pool(name="inp", bufs=3))
        zpool = ctx.enter_context(tc.tile_pool(name="z", bufs=2))
        opool = ctx.enter_context(tc.tile_pool(name="o", bufs=3))
        pp1 = ctx.enter_context(tc.tile_pool(name="pp1", bufs=2, space="PSUM"))
        pp2 = ctx.enter_context(tc.tile_pool(name="pp2", bufs=1, space="PSUM"))
        pp3 = ctx.enter_context(tc.tile_pool(name="pp3", bufs=2, space="PSUM"))

        ones_col = cpool.tile([1, 128], bf)
        nc.vector.memset(ones_col[:], 1.0)

        for m in range(nm):
            xw = inp.tile([12, 2 * XWU], f8, tag="xw")
            # 3-way chunked dst AP -> 36 descriptors across the 16 SDMA engines
            xw_dst = bass.AP(xw[:].tensor, xw[:].offset,
                             [list(xw[:].ap[0]), [2 * XWU // 3, 3], [1, 2 * XWU // 3]])
            xw_src = bass.AP(xw_d.ap()[m].tensor, xw_d.ap()[m].offset,
                             [list(xw_d.ap()[m].ap[0]), [2 * XWU // 3, 3], [1, 2 * XWU // 3]])
            nc.sync.dma_start(xw_dst, xw_src)
            wb = inp.tile([128, 2 * WBU], bf, tag="wb")
            nc.sync.dma_start(wb[:], wb_d.ap()[m])

            probs = opool.tile([128, 140], bf, tag="probs")
            for u in range(2):
                xo = u * XWU
                wo = u * WBU
                # L1: fp8 DoubleRow, k-tiles = hi/lo residual halves of W1
                p1 = pp1.tile([128, 896], f32, tag="p1")
                for q in range(Q):
                    st = xw[:, xo + 896 + q * 256: xo + 896 + (q + 1) * 256]
                    lhs = bass.AP(st.tensor, st.offset,
                                  [list(st.ap[0]), [128, 2], [1, 128]])
                    mv = xw[:, xo + q * 128: xo + (q + 1) * 128]
                    rhs = bass.AP(mv.tensor, mv.offset,
                                  [list(mv.ap[0]), [0, 2], [1, 128]])
                    nc.tensor.matmul(p1[:, q * 128:(q + 1) * 128], lhs, rhs,
                                     start=True, stop=True, perf_mode=DR)
                z1 = zpool.tile([128, 896], bf, tag="z1")
                nc.vector.tensor_scalar_max(z1[:], p1[:], 0.0)

                # L2: one shared block-diag stationary, 2 wide matmuls
                p2 = pp2.tile([128, 896], f32, tag="p2")
                nc.tensor.matmul(p2[:, 0:512], wb[:, wo:wo + 128],
                                 z1[:, 0:512], start=True, stop=True)
                nc.tensor.matmul(p2[:, 512:896], wb[:, wo:wo + 128],
                                 z1[:, 512:896], start=True, stop=True)
                z2 = zpool.tile([128, 896], bf, tag="z2")
                b2ap = wb[:, wo + 198:wo + 200].bitcast(f32)
                nc.scalar.activation(z2[:], p2[:], AF.Relu, bias=b2ap, scale=1.0)

                # L3: bias seed + per-question products (z2 stationary)
                p3 = pp3.tile([128, 70], f32, tag="p3")
                nc.tensor.matmul(p3[:], ones_col[:], wb[0:1, wo + 200:wo + 270],
                                 start=True, stop=False)
                for q in range(Q):
                    nc.tensor.matmul(
                        p3[:, q * 10:(q + 1) * 10],
                        z2[:, q * 128:(q + 1) * 128],
                        wb[:, wo + 128 + q * 10: wo + 138 + q * 10],
                        start=False, stop=(q == Q - 1))
                # logits psum -> bf16 SBUF (alternate engines per sub-tile)
                dst = probs[:, u * 70:(u + 1) * 70]
                if u == 0:
                    nc.vector.tensor_scalar_add(dst, p3[:], 0.0)
                else:
                    nc.scalar.copy(dst, p3[:])

            eng = nc.sync if m == nm - 1 else nc.gpsimd
            eng.dma_start(out_d.ap()[m], probs[:])

    nc.compile()
    return nc


def _get_program(teff=T):
    if teff not in _cache:
        _cache[teff] = _build_program(teff)
    return _cache[teff]


# ----------------------------------------------------------------------------
# host-side data prep
# ----------------------------------------------------------------------------

def _expert_blobs(W1, W1_a, W2, W2_a, V, V_a):
    """Per-judge packed weight blocks.

    Returns wa8 [J, 12, 7*256] fp8 (L1 hi/lo stationaries) and
    wb16 [J, 128, WBU] uint16 (bf16 bits; W2 block-diag, V blocks, b2 f32
    bits, b3 row)."""
    W1c = (W1[None] + W1_a).astype(np.float32)    # [J,Q,H,O+1]
    W2c = (W2[None] + W2_a).astype(np.float32)    # [J,H,H+1]
    Vc = (V[None] + V_a).astype(np.float32)       # [J,Q,O,H+1]

    # fold the x -0.5 shift into the bias column, then hi/lo fp8 split
    W1m = W1c.copy()
    W1m[..., 0] = W1c[..., 0] + 0.5 * W1c[..., 1:].sum(-1)
    hi = W1m.astype(_f8)
    lo = (W1m - hi.astype(np.float32)).astype(_f8)

    wa8 = np.zeros((J, 12, Q, 2, 128), _f8)
    hit = hi.transpose(0, 3, 1, 2)   # [J, d, q, h]
    lot = lo.transpose(0, 3, 1, 2)
    for par in range(2):
        wa8[:, par * 6:(par + 1) * 6, :, 0, par * 64:(par + 1) * 64] = hit
        wa8[:, par * 6:(par + 1) * 6, :, 1, par * 64:(par + 1) * 64] = lot
    wa8 = wa8.reshape(J, 12, Q * 256)

    wb = np.zeros((J, 128, WBU), np.float32)
    w2t = W2c[:, :, 1:].transpose(0, 2, 1)        # [J, i(h1), h2]
    for par in range(2):
        s = slice(par * 64, (par + 1) * 64)
        wb[:, s, s] = w2t
        for q in range(Q):
            # V block: rows (par,h) -> cols 128 + q*10 + par*5 + o
            wb[:, s, 128 + q * 10 + par * 5: 128 + q * 10 + par * 5 + 5] = \
                Vc[:, q, :, 1:].transpose(0, 2, 1)
    wb16 = np.zeros((J, 128, WBU), np.uint16)
    wb16[:, :, :198] = wb[:, :, :198].astype(_bf16).view(np.uint16)
    # b2 as raw f32 bits in cols 198:200
    b2 = np.concatenate([W2c[:, :, 0], W2c[:, :, 0]], axis=1)   # [J, 128]
    wb16[:, :, 198:200] = b2.astype(np.float32).view(np.uint16).reshape(J, 128, 2)
    # b3 row on partition 0, cols 200:270: col q*10 + par*5 + o
    b3 = np.stack([Vc[:, :, :, 0]] * 2, axis=2).reshape(J, Q * 10)  # [J,q,par,o]
    wb16[:, 0, 200:270] = b3.astype(_bf16).view(np.uint16)
    return wa8, wb16


def _plan(judge_ids):
    """Supertile schedule: list of (judge, sample_idx_array)."""
    jid = np.asarray(judge_ids).astype(np.int64).ravel()
    assert jid.shape[0] == B
    order = np.argsort(jid, kind="stable")
    counts = np.bincount(jid, minlength=J)
    tiles = []
    pos = 0
    for j in range(J):
        g = order[pos:pos + counts[j]]
        pos += counts[j]
        for s in range(0, len(g), ST):
            tiles.append((j, g[s:s + ST]))
    assert len(tiles) <= N_CORES * T, f"{len(tiles)} supertiles > capacity"
    return tiles


def _prepare_inputs(x, judge_ids, W1, W1_a, W2, W2_a, V, V_a):
    x = np.ascontiguousarray(np.asarray(x, dtype=np.float32))
    wa8, wb16 = _expert_blobs(*(np.asarray(a, dtype=np.float32)
                                for a in (W1, W1_a, W2, W2_a, V, V_a)))
    tiles = _plan(judge_ids)
    teff = -(-len(tiles) // N_CORES)
    teff += teff % 2

    judge_mat = np.zeros((N_CORES, teff), np.int64)
    xg = np.full((N_CORES, teff, ST, Q, O), 0.5, np.float32)
    for i, (j, g) in enumerate(tiles):
        k, t = i % N_CORES, i // N_CORES
        judge_mat[k, t] = j
        xg[k, t, :len(g)] = x[g]

    # x -> fp8, shifted; layout [12 part = (par, d), 896 = (q, pair)]
    xs8 = (xg - 0.5).astype(_f8)                       # [NC, teff, 256, Q, O]
    xs8 = xs8.reshape(N_CORES, teff, NP, 2, Q, O)      # [..., pair, par, q, d]
    xt8 = np.zeros((N_CORES, teff, 12, Q, NP), _f8)
    one8 = np.float32(1.0).astype(_f8)
    xt8[:, :, 0] = one8
    xt8[:, :, 6] = one8
    for par in range(2):
        # [NC, teff, pair, q, d] -> [NC, teff, d, q, pair]
        blk = xs8[:, :, :, par].transpose(0, 1, 4, 3, 2)
        xt8[:, :, par * 6 + 1: par * 6 + 6] = blk
    xt8 = xt8.reshape(N_CORES, teff, 12, Q * NP)

    nm = teff // 2
    in_maps = []
    for k in range(N_CORES):
        xwm = np.zeros((teff, 12, XWU), _f8)
        xwm[:, :, :896] = xt8[k]
        xwm[:, :, 896:] = wa8[judge_mat[k]]
        wbm = wb16[judge_mat[k]]                       # [teff, 128, WBU]
        in_maps.append({
            "xw": np.ascontiguousarray(
                xwm.reshape(nm, 2, 12, XWU).transpose(0, 2, 1, 3)),
            "wb": np.ascontiguousarray(
                wbm.reshape(nm, 2, 128, WBU).transpose(0, 2, 1, 3)
                .reshape(nm, 128, 2 * WBU)).view(_bf16),
        })
    return in_maps, tiles, teff


def _assemble_output(results, tiles, teff):
    logits = np.empty((B, Q, O), np.float32)
    nm = teff // 2
    for k in range(N_CORES):
        blob = np.asarray(results[k]["out"]).view(np.uint16)
        # [nm, 128, 140] -> [teff, 128, 70]
        blob = blob.reshape(nm, 128, 2, 70).transpose(0, 2, 1, 3) \
                   .reshape(teff, 128, 70)
        f32 = (blob.astype(np.uint32) << 16).view(np.float32)
        for i, (_, g) in enumerate(tiles):
            if i % N_CORES != k:
                continue
            t = i // N_CORES
            # [128 pairs, 70=(q,par,o)] -> [256 samples, Q, O]
            rows = f32[t].reshape(128, Q, 2, O).transpose(0, 2, 1, 3) \
                         .reshape(ST, Q, O)
            logits[g] = rows[:len(g)]
    m = logits.max(-1, keepdims=True)
    e = np.exp(logits - m)
    return (e / e.sum(-1, keepdims=True)).astype(np.float32)


# ----------------------------------------------------------------------------
# entry point
# ----------------------------------------------------------------------------

def kernel(x, judge_ids, W1, W1_a, W2, W2_a, V, V_a):
    from concourse import bass_utils
    in_maps, tiles, teff = _prepare_inputs(x, judge_ids, W1, W1_a, W2, W2_a, V, V_a)
    nc = _get_program(teff)
    res = bass_utils.run_bass_kernel_spmd(
        nc, in_maps, core_ids=list(range(N_CORES)), trace=False)
    return _assemble_output(res.results, tiles, teff)


# expose for test harness reuse
def run_with_results(x, judge_ids, W1, W1_a, W2, W2_a, V, V_a, trace=False,
                     **kwargs):
    from concourse import bass_utils
    in_maps, tiles, teff = _prepare_inputs(x, judge_ids, W1, W1_a, W2, W2_a, V, V_a)
    nc = _get_program(teff)
    res = bass_utils.run_bass_kernel_spmd(
        nc, in_maps, core_ids=list(range(N_CORES)), trace=trace, **kwargs)
    return _assemble_output(res.results, tiles, teff), res



# revision 10
# speedup vs baseline: 1.1880x; 1.1880x over previous
"""Trainium2 Bass kernel for nn_CalibrationNetwork (MoE-routed 3-layer MLP + softmax).

Strategy (v3): judge-contiguous scheduling
------------------------------------------
Host sorts samples by judge. The 32 judges are ranked by size and snake-
assigned to 8 cores x 4 slots; slot k has a common (max-padded) pair count
across cores so one SPMD program serves all cores. Per-judge combined
weights (W1+W1_a etc.) are packed per slot; all matmuls then use N=512
moving columns so LDWEIGHTS hides in the PE reorder window.

Layouts (parity-pair packing, 2 samples per column everywhere):
  z partition   p = par*64 + h           (par = sample parity in its pair)
  L1: stationary per q = [12 rows=(par,d0..5), 128 cols=(par,h)] block-diag,
      7 q's stacked on disjoint row ranges QROW[q] (2 per 32-strip) so all
      share one 128-col block; x lives on the same rows. Row-tiled matmuls
      (tile_position=(32g,0)) run concurrently across strips.
  L2: stationary [128=(par,h1), 128=(par,h2)] block-diag; bias b2 applied
      by the ACT relu2 drain (f32 bias columns).
  L3: stationary per q = V block [128=(par,h2), 10=(par,o)], col-tiled
      (tile_position=(0,32g)); output partitions 32g+par*5+o.
  V/b3 bias and softmax are applied on the HOST (judge known per sample).

Pipeline: chunks of <=512 pairs flow L1 -> relu1(DVE) -> L2 -> relu2(ACT)
-> L3 -> copy(ACT) with software-pipelined emission (L1 of chunk i+1 is
emitted before L2 of chunk i, etc.) and two 2-bank PSUM tags as the
pipeline buffer. A short warmup matmul burst trips the PE HAM throttle
to full clock while the input DMAs land.
"""

import numpy as np
import ml_dtypes

B, J, Q, O, H = 32768, 32, 7, 5, 64
N_CORES = 8
NSLOTS = J // N_CORES          # 4 judges (slots) per core
CHUNK = 512                    # max pairs per matmul
PADP = 32                      # slot pair counts padded to multiple of this
WJ = 456                       # weight cols/slot: 2x128 L1 + 128 L2 + 70 V + 2 b2
QROW = (0, 32, 64, 96, 0, 32, 64)    # x/W1 row base per question (32-aligned)
XB_OF_Q = (0, 0, 0, 0, 1, 1, 1)      # x/W1 column block per question
G_OF_Q = (0, 1, 2, 3, 0, 1, 2)       # 32-strip (row/col group) per question
WARMUP_MMS = 7

_bf16 = ml_dtypes.bfloat16
_cache = {}


def _chunks(S):
    """Split S pairs into matmul chunks (512s + one ragged multiple of 32)."""
    out = [CHUNK] * (S // CHUNK)
    if S % CHUNK:
        out.append(S % CHUNK)
    return out


# ----------------------------------------------------------------------------
# device program
# ----------------------------------------------------------------------------

def _build_program(slots):
    import concourse.bacc as bacc
    import concourse.tile as tile
    import concourse.mybir as mybir
    import concourse.bass as bass
    from contextlib import ExitStack

    bf = mybir.dt.bfloat16
    f32 = mybir.dt.float32
    AF = mybir.ActivationFunctionType

    TP = sum(slots)
    offs = np.cumsum([0] + list(slots))[:-1]

    nc = bacc.Bacc("TRN2", target_bir_lowering=False, debug=False)
    xa_d = nc.dram_tensor("xa", (128, 2 * TP), bf, kind="ExternalInput")
    wt_d = nc.dram_tensor("wt", (128, NSLOTS * WJ), bf, kind="ExternalInput")
    out_d = nc.dram_tensor("out", (128, 2 * TP), bf, kind="ExternalOutput")

    with ExitStack() as ctx:
        tc = ctx.enter_context(tile.TileContext(nc))
        cpool = ctx.enter_context(tc.tile_pool(name="const", bufs=1))
        ppool = ctx.enter_context(tc.tile_pool(name="ps", bufs=2, space="PSUM"))

        xa_t = cpool.tile([128, 2 * TP], bf)
        wt_t = cpool.tile([128, NSLOTS * WJ], bf)
        z1 = cpool.tile([128, 7 * TP], bf)
        z2 = cpool.tile([128, 7 * TP], bf)
        lg = cpool.tile([128, 2 * TP], bf)
        warm = cpool.tile([1, 640], bf)

        nc.vector.memset(warm[:], 1.0)
        # weights first (small), then x per slot so L1 of slot 0 starts early
        nc.sync.dma_start(wt_t[:], wt_d.ap())
        for s in range(NSLOTS):
            o = int(offs[s])
            for xb in range(2):
                nc.sync.dma_start(
                    xa_t[:, xb * TP + o:xb * TP + o + slots[s]],
                    xa_d.ap()[:, xb * TP + o:xb * TP + o + slots[s]])

        # PE warmup: trip HAM to 8/8 while DMAs land
        wtile = ppool.tile([128, 1024], f32, tag="pa")
        for _ in range(WARMUP_MMS):
            nc.tensor.matmul(wtile[:, 0:512], warm[0:1, 0:128], warm[0:1, 128:640],
                             start=True, stop=True)

        # chunk sequence: (slot, judge-col-base, pair0, npairs, z1colbase, lgbase)
        seq = []
        for s in range(NSLOTS):
            o = int(offs[s])
            p0 = 0
            for n in _chunks(slots[s]):
                seq.append((s, p0, n))
                p0 += n

        def l1(i):
            s, p0, n = seq[i]
            o = int(offs[s])
            wc = s * WJ
            zb = 7 * o + 7 * p0
            # rounds of 2 questions -> one 2-bank tile each
            for r, qs in enumerate(((0, 1), (2, 3), (4, 5), (6,))):
                t = ppool.tile([128, 1024], f32, tag="pa", name=f"p1_{i}_{r}")
                for k, q in enumerate(qs):
                    rw = QROW[q]
                    xb = XB_OF_Q[q]
                    nc.tensor.matmul(
                        t[:, 512 * k:512 * k + n],
                        wt_t[rw:rw + 12, wc + 128 * xb:wc + 128 * xb + 128],
                        xa_t[rw:rw + 12, xb * TP + o + p0:xb * TP + o + p0 + n],
                        start=True, stop=True,
                        tile_position=(32 * G_OF_Q[q], 0))
                dst = z1[:, zb + 2 * r * n: zb + 2 * r * n + len(qs) * n]
                if len(qs) == 2 and n < 512:
                    src = bass.AP(t[:].tensor, t[:].offset,
                                  [list(t[:].ap[0]), [512, 2], [1, n]])
                    dst = bass.AP(dst.tensor, dst.offset,
                                  [list(dst.ap[0]), [n, 2], [1, n]])
                else:
                    src = t[:, 0:len(qs) * 512] if n == 512 else t[:, 0:n]
                nc.vector.tensor_scalar_max(dst, src, 0.0)

        def l2(i):
            s, p0, n = seq[i]
            o = int(offs[s])
            wc = s * WJ
            zb = 7 * o + 7 * p0
            b2ap = wt_t[:, wc + 454:wc + 456].bitcast(f32)
            done = 0
            while done < 7 * n:
                piece = min(1024, 7 * n - done)
                t = ppool.tile([128, 1024], f32, tag="pb", name=f"p2_{i}_{done}")
                na = min(512, piece)
                nc.tensor.matmul(t[:, 0:na], wt_t[:, wc + 256:wc + 384],
                                 z1[:, zb + done:zb + done + na],
                                 start=True, stop=True)
                if piece > 512:
                    nc.tensor.matmul(t[:, 512:piece], wt_t[:, wc + 256:wc + 384],
                                     z1[:, zb + done + 512:zb + done + piece],
                                     start=True, stop=True)
                if piece <= 512 or piece == 1024:
                    # contiguous in both PSUM and z2
                    nc.scalar.activation(z2[:, zb + done:zb + done + piece],
                                         t[:, 0:piece], AF.Relu, bias=b2ap, scale=1.0)
                else:
                    # unequal halves: two drains
                    nc.scalar.activation(z2[:, zb + done:zb + done + 512],
                                         t[:, 0:512], AF.Relu, bias=b2ap, scale=1.0)
                    nc.scalar.activation(z2[:, zb + done + 512:zb + done + piece],
                                         t[:, 512:piece], AF.Relu, bias=b2ap, scale=1.0)
                done += piece

        def l3(i):
            s, p0, n = seq[i]
            o = int(offs[s])
            wc = s * WJ
            zb = 7 * o + 7 * p0
            lgb = 2 * o + 2 * p0
            t = ppool.tile([128, 1024], f32, tag="pb", name=f"p3_{i}")
            for q in range(7):
                g = G_OF_Q[q]
                rnd = 0 if q < 4 else 1
                nc.tensor.matmul(
                    t[32 * g:32 * g + 10, 512 * rnd:512 * rnd + n],
                    wt_t[:, wc + 384 + 10 * q:wc + 394 + 10 * q],
                    z2[:, zb + q * n:zb + (q + 1) * n],
                    start=True, stop=True,
                    tile_position=(0, 32 * g))
            if n == 512:
                nc.scalar.copy(lg[:, lgb:lgb + 1024], t[:, 0:1024])
            else:
                src = bass.AP(t[:].tensor, t[:].offset,
                              [list(t[:].ap[0]), [512, 2], [1, n]])
                d = lg[:, lgb:lgb + 2 * n]
                dst = bass.AP(d.tensor, d.offset,
                              [list(d.ap[0]), [n, 2], [1, n]])
                nc.scalar.copy(dst, src)

        # software-pipelined emission with per-seg output DMA
        last_chunk_of_slot = {}
        for i, (s, p0, n) in enumerate(seq):
            last_chunk_of_slot[s] = i
        nseq = len(seq)
        for i in range(nseq + 2):
            if i < nseq:
                l1(i)
            if 1 <= i <= nseq:
                l2(i - 1)
            if 2 <= i:
                j = i - 2
                l3(j)
                s = seq[j][0]
                if last_chunk_of_slot[s] == j:
                    o = int(offs[s])
                    eng = nc.sync if s == NSLOTS - 1 else nc.gpsimd
                    eng.dma_start(out_d.ap()[:, 2 * o:2 * o + 2 * slots[s]],
                                  lg[:, 2 * o:2 * o + 2 * slots[s]])

    nc.compile()
    return nc


def _get_program(slots):
    key = tuple(slots)
    if key not in _cache:
        _cache[key] = _build_program(key)
    return _cache[key]


# ----------------------------------------------------------------------------
# host-side scheduling and packing
# ----------------------------------------------------------------------------

def _schedule(judge_ids):
    """Snake-assign judges to (core, slot); returns slot sizes + per-core judge
    lists + per-judge sample index arrays (sorted order)."""
    jid = np.asarray(judge_ids).astype(np.int64).ravel()
    assert jid.shape[0] == B
    order = np.argsort(jid, kind="stable")
    counts = np.bincount(jid, minlength=J)
    pos = np.cumsum([0] + list(counts))
    samples = [order[pos[j]:pos[j + 1]] for j in range(J)]
    pairs = np.array([(c + 1) // 2 for c in counts])

    rank = np.argsort(-pairs, kind="stable")
    slots = []
    assign = np.zeros((N_CORES, NSLOTS), np.int64)   # judge id per (core, slot)
    for s in range(NSLOTS):
        grp = rank[8 * s:8 * s + 8]
        size = int(-(-max(1, pairs[grp].max()) // PADP) * PADP)
        slots.append(size)
        for k in range(N_CORES):
            assign[k, s] = grp[k]
    return tuple(slots), assign, samples, pairs


def _pack_inputs(x, judge_ids, W1c, W2c, Vc, slots, assign, samples):
    TP = sum(slots)
    offs = np.cumsum([0] + list(slots))[:-1]
    x = np.asarray(x, np.float32)

    # weights per judge, packed per (core, slot)
    wtj = np.zeros((J, 128, WJ), np.float32)
    for q in range(Q):
        rw = QROW[q]
        xb = XB_OF_Q[q]
        for par in range(2):
            # L1 block: rows rw+par*6+d, cols 128*xb + par*64+h
            blk = W1c[:, q].transpose(0, 2, 1)          # [J, d, h]
            wtj[:, rw + 6 * par:rw + 6 * par + 6,
                128 * xb + 64 * par:128 * xb + 64 * par + 64] = blk
    for par in range(2):
        s = slice(64 * par, 64 * par + 64)
        wtj[:, s, 256 + 64 * par:256 + 64 * par + 64] = \
            W2c[:, :, 1:].transpose(0, 2, 1)            # rows h1, cols h2
        for q in range(Q):
            wtj[:, s, 384 + 10 * q + 5 * par:384 + 10 * q + 5 * par + 5] = \
                Vc[:, q, :, 1:].transpose(0, 2, 1)      # rows h2, cols o
    wt16 = np.zeros((J, 128, WJ), np.uint16)
    wt16[:, :, :454] = wtj[:, :, :454].astype(_bf16).view(np.uint16)
    b2 = np.concatenate([W2c[:, :, 0], W2c[:, :, 0]], axis=1)   # [J, 128]
    wt16[:, :, 454:456] = b2.astype(np.float32).view(np.uint16).reshape(J, 128, 2)

    in_maps = []
    for k in range(N_CORES):
        xa = np.zeros((128, 2 * TP), np.float32)
        wt = np.zeros((128, NSLOTS * WJ), np.uint16)
        for s in range(NSLOTS):
            j = int(assign[k, s])
            o = int(offs[s])
            g = samples[j]
            wt[:, s * WJ:(s + 1) * WJ] = wt16[j]
            for par in range(2):
                gs = g[par::2]
                ns = len(gs)
                xv = x[gs]                               # [ns, Q, O]
                for q in range(Q):
                    rw = QROW[q] + 6 * par
                    cb = XB_OF_Q[q] * TP + o
                    xa[rw, cb:cb + ns] = 1.0
                    xa[rw + 1:rw + 6, cb:cb + ns] = xv[:, q, :].T
        in_maps.append({"xa": xa.astype(_bf16),
                        "wt": wt.view(_bf16)})
    return in_maps


def _unpack_output(results, judge_ids, b3, slots, assign, samples):
    TP = sum(slots)
    offs = np.cumsum([0] + list(slots))[:-1]
    logits = np.zeros((B, Q, O), np.float32)
    for k in range(N_CORES):
        blob = np.asarray(results[k]["out"]).view(np.uint16)
        f = (blob.astype(np.uint32) << 16).view(np.float32)   # [128, 2*TP]
        for s in range(NSLOTS):
            j = int(assign[k, s])
            o = int(offs[s])
            g = samples[j]
            S = len(g)
            p0 = 0
            for n in _chunks(slots[s]):
                for q in range(Q):
                    gq = G_OF_Q[q]
                    rnd = 0 if q < 4 else 1
                    cb = 2 * o + 2 * p0 + rnd * n
                    for par in range(2):
                        rows = slice(32 * gq + 5 * par, 32 * gq + 5 * par + 5)
                        idx = 2 * (p0 + np.arange(n)) + par
                        v = idx < S
                        if v.any():
                            logits[g[idx[v]], q, :] = f[rows, cb:cb + n].T[v]
                p0 += n
    logits += b3[np.asarray(judge_ids).astype(np.int64).ravel()]
    m = logits.max(-1, keepdims=True)
    e = np.exp(logits - m)
    return (e / e.sum(-1, keepdims=True)).astype(np.float32)


def _prepare(x, judge_ids, W1, W1_a, W2, W2_a, V, V_a):
    W1c = (np.asarray(W1, np.float32)[None] + np.asarray(W1_a, np.float32))
    W2c = (np.asarray(W2, np.float32)[None] + np.asarray(W2_a, np.float32))
    Vc = (np.asarray(V, np.float32)[None] + np.asarray(V_a, np.float32))
    b3 = Vc[:, :, :, 0]                                  # [J, Q, O]
    slots, assign, samples, pairs = _schedule(judge_ids)
    in_maps = _pack_inputs(x, judge_ids, W1c, W2c, Vc, slots, assign, samples)
    return in_maps, (judge_ids, b3, slots, assign, samples)


# ----------------------------------------------------------------------------
# entry points
# ----------------------------------------------------------------------------

def kernel(x, judge_ids, W1, W1_a, W2, W2_a, V, V_a):
    from concourse import bass_utils
    in_maps, meta = _prepare(x, judge_ids, W1, W1_a, W2, W2_a, V, V_a)
    nc = _get_program(meta[2])
    res = bass_utils.run_bass_kernel_spmd(
        nc, in_maps, core_ids=list(range(N_CORES)), trace=False)
    return _unpack_output(res.results, meta[0], meta[1], meta[2], meta[3], meta[4])


def run_with_results(x, judge_ids, W1, W1_a, W2, W2_a, V, V_a, trace=False,
                     **kwargs):
    from concourse import bass_utils
    in_maps, meta = _prepare(x, judge_ids, W1, W1_a, W2, W2_a, V, V_a)
    nc = _get_program(meta[2])
    res = bass_utils.run_bass_kernel_spmd(
        nc, in_maps, core_ids=list(range(N_CORES)), trace=trace, **kwargs)
    out = _unpack_output(res.results, meta[0], meta[1], meta[2], meta[3], meta[4])
    return out, res
